# revision 5
# baseline (speedup 1.0000x reference)
"""MoE expert-parallel FFN kernel for Trainium2 (8 NeuronCores).

Problem: per-expert GEMM -> ReLU -> per-expert GEMM
  dispatched_input: (E=8, C=2048, M=2048) f32
  inner_experts:    (E=8, M=2048, H=8192) f32
  out_experts:      (E=8, H=8192, M=2048) f32
  out:              (E=8, C=2048, M=2048) f32

Sharding: pure expert parallelism — expert e runs entirely on core e.
No collectives needed.

Per-core dataflow (bf16 compute, fp32 PSUM accumulation):
  Phase 0: transpose X (C,M) -> X^T (M,C) resident in SBUF as bf16
           (TensorE 128x128 transposes, DVE cast/evict).
  Phase 1: actT[h,c] = relu(W1^T X^T) streamed over h, accumulated over
           m in PSUM. W1 streamed once; ReLU+cast on ScalarE; actT
           spilled to DRAM scratch as bf16 (32MB).
  Phase 2: Y[c,m] = actT^T @ W2 accumulated over h in PSUM. actT
           c-strips cached in SBUF (16MB); W2 streamed twice.
"""

import numpy as np

import concourse.bass as bass
import concourse.tile as tile
from concourse import bacc, mybir
from concourse.bass_utils import run_bass_kernel_spmd
from concourse.masks import make_identity

E = 8
C = 2048  # tokens per expert
M = 2048  # model dim
H = 8192  # ffn dim
P = 128   # partitions
FD = 512  # matmul moving free dim (one PSUM bank of fp32)

BF = mybir.dt.bfloat16
F32 = mybir.dt.float32

MT = M // P   # 16 m-tiles
CT = C // P   # 16 c-tiles
HT = H // P   # 64 h-tiles

_CACHED_NC = None


def _build_nc():
    nc = bacc.Bacc(
        "TRN2",
        target_bir_lowering=False,
        debug=False,
        num_devices=E,
    )
    x = nc.declare_dram_parameter("dispatched_input", [C, M], F32, isOutput=False)
    w1 = nc.declare_dram_parameter("inner_experts", [M, H], F32, isOutput=False)
    w2 = nc.declare_dram_parameter("out_experts", [H, M], F32, isOutput=False)
    y = nc.declare_dram_parameter("out", [C, M], F32, isOutput=True)

    with tile.TileContext(nc) as tc:
        with tc.tile_pool(name="dram", bufs=1, space="DRAM") as dram_pool:
            # actT[h, c] = relu(X @ W1)^T spill buffer
            actT = dram_pool.tile([H, C], BF)

            with tc.tile_pool(name="xT", bufs=1) as xT_pool:
                # X^T resident: column block mt*C..(mt+1)*C holds
                # X^T[128 m of tile mt, all c]
                xT = xT_pool.tile([P, MT * C], BF)

                # ---- Phase 0: X -> X^T (bf16) ----
                with tc.tile_pool(name="xstage", bufs=3) as xs_pool, \
                     tc.tile_pool(name="xbf", bufs=3) as xb_pool, \
                     tc.tile_pool(name="tpsum", bufs=8, space="PSUM") as tp_pool, \
                     tc.tile_pool(name="ident", bufs=1) as id_pool:
                    ident = id_pool.tile([P, P], BF)
                    make_identity(nc, ident)
                    for ct in range(CT):
                        xs = xs_pool.tile([P, M], F32, tag="xs")
                        nc.sync.dma_start(xs[:], x[ct * P:(ct + 1) * P, :])
                        xb = xb_pool.tile([P, M], BF, tag="xb")
                        nc.vector.tensor_copy(xb[:], xs[:])
                        for mt in range(MT):
                            tp = tp_pool.tile([P, P], BF, tag="tp")
                            nc.tensor.transpose(
                                tp[:], xb[:, mt * P:(mt + 1) * P], ident[:])
                            nc.vector.tensor_copy(
                                xT[:, mt * C + ct * P: mt * C + (ct + 1) * P],
                                tp[:])

                # ---- Phase 1: actT = relu(W1.T @ X.T), stream W1 once ----
                HS = 512          # h panel width staged at a time
                NHS = H // HS     # 16
                with tc.tile_pool(name="w1s", bufs=4) as w1s_pool, \
                     tc.tile_pool(name="w1b", bufs=2 * MT) as w1b_pool, \
                     tc.tile_pool(name="ps1", bufs=8, space="PSUM") as ps1_pool, \
                     tc.tile_pool(name="acts", bufs=4) as act_pool:
                    for hs in range(NHS):
                        w1b_tiles = []
                        for mt in range(MT):
                            ws = w1s_pool.tile([P, HS], F32, tag="w1s")
                            nc.sync.dma_start(
                                ws[:],
                                w1[mt * P:(mt + 1) * P, hs * HS:(hs + 1) * HS])
                            wb = w1b_pool.tile([P, HS], BF, tag="w1b")
                            nc.vector.tensor_copy(wb[:], ws[:])
                            w1b_tiles.append(wb)
                        for hb in range(HS // P):  # 4 h-blocks of 128
                            pss = [ps1_pool.tile([P, FD], F32, tag="ps1", name=f"ps1_{hs}_{hb}_{i}")
                                   for i in range(C // FD)]
                            for mt in range(MT):
                                lhsT = w1b_tiles[mt][:, hb * P:(hb + 1) * P]
                                for cc in range(C // FD):
                                    nc.tensor.matmul(
                                        pss[cc][:],
                                        lhsT,
                                        xT[:, mt * C + cc * FD: mt * C + (cc + 1) * FD],
                                        start=(mt == 0),
                                        stop=(mt == MT - 1),
                                    )
                            at = act_pool.tile([P, C], BF, tag="acts")
                            for cc in range(C // FD):
                                nc.scalar.activation(
                                    at[:, cc * FD:(cc + 1) * FD],
                                    pss[cc][:],
                                    mybir.ActivationFunctionType.Relu,
                                )
                            h0 = hs * HS + hb * P
                            nc.sync.dma_start(actT[h0:h0 + P, :], at[:])

            # ---- Phase 2: Y = actT.T @ W2, c-strips cached ----
            CS = 1024         # c-strip cached in SBUF
            NCS = C // CS     # 2
            MC = 512          # m chunk (one PSUM bank)
            NMC = M // MC     # 4
            with tc.tile_pool(name="aT", bufs=1) as aT_pool, \
                 tc.tile_pool(name="w2s", bufs=4) as w2s_pool, \
                 tc.tile_pool(name="w2b", bufs=4) as w2b_pool, \
                 tc.tile_pool(name="ps2", bufs=8, space="PSUM") as ps2_pool, \
                 tc.tile_pool(name="ostage", bufs=8) as o_pool:
                for cs in range(NCS):
                    # aT column block ht*CS..(ht+1)*CS = actT[h-tile ht, c strip]
                    aT = aT_pool.tile([P, HT * CS], BF, tag="aT")
                    for ht in range(HT):
                        nc.sync.dma_start(
                            aT[:, ht * CS:(ht + 1) * CS],
                            actT[ht * P:(ht + 1) * P, cs * CS:(cs + 1) * CS])
                    for mc in range(NMC):
                        pcs = [ps2_pool.tile([P, MC], F32, tag="ps2", name=f"ps2_{cs}_{mc}_{i}")
                               for i in range(CS // P)]
                        for ht in range(HT):
                            ws = w2s_pool.tile([P, MC], F32, tag="w2s")
                            nc.sync.dma_start(
                                ws[:],
                                w2[ht * P:(ht + 1) * P, mc * MC:(mc + 1) * MC])
                            wb = w2b_pool.tile([P, MC], BF, tag="w2b")
                            nc.vector.tensor_copy(wb[:], ws[:])
                            for ct in range(CS // P):
                                nc.tensor.matmul(
                                    pcs[ct][:],
                                    aT[:, ht * CS + ct * P: ht * CS + (ct + 1) * P],
                                    wb[:],
                                    start=(ht == 0),
                                    stop=(ht == HT - 1),
                                )
                        for ct in range(CS // P):
                            ob = o_pool.tile([P, MC], F32, tag="ostage")
                            nc.vector.tensor_copy(ob[:], pcs[ct][:])
                            c0 = cs * CS + ct * P
                            nc.sync.dma_start(
                                y[c0:c0 + P, mc * MC:(mc + 1) * MC], ob[:])
    nc.compile()
    return nc


def get_nc():
    global _CACHED_NC
    if _CACHED_NC is None:
        _CACHED_NC = _build_nc()
    return _CACHED_NC


def kernel(dispatched_input, inner_experts, out_experts):
    dispatched_input = np.ascontiguousarray(dispatched_input, dtype=np.float32)
    inner_experts = np.ascontiguousarray(inner_experts, dtype=np.float32)
    out_experts = np.ascontiguousarray(out_experts, dtype=np.float32)
    assert dispatched_input.shape == (E, C, M)
    assert inner_experts.shape == (E, M, H)
    assert out_experts.shape == (E, H, M)

    nc = get_nc()
    in_maps = [
        {
            "dispatched_input": dispatched_input[e],
            "inner_experts": inner_experts[e],
            "out_experts": out_experts[e],
        }
        for e in range(E)
    ]
    res = run_bass_kernel_spmd(nc, in_maps, core_ids=list(range(E)))
    return np.stack([res.results[e]["out"] for e in range(E)], axis=0)
